# revision 8
# baseline (speedup 1.0000x reference)
"""Trainium2 Bass kernel for a single-step LSTM cell (NaiveLSTM).

Reference computation (fp32):
    x: [2048, 4096] (input_size, batch)
    h, c: [4096, 2048] (batch, hidden)
    i = sigmoid(w_ii @ x + b_ii + w_hi @ h.T + b_hi)
    f = sigmoid(w_if @ x + b_if + w_hf @ h.T + b_hf)
    g = tanh   (w_ig @ x + b_ig + w_hg @ h.T + b_hg)
    o = sigmoid(w_io @ x + b_io + w_ho @ h.T + b_ho)
    c' = f * c.T + i * g ; h' = o * tanh(c')
    returns (h'.T, c'.T), each [4096, 2048]

Distribution: tensor-parallel over the hidden dimension. Each of the 8
cores owns 256 output hidden rows: its shard of all 8 weight matrices
(pre-transposed on host into matmul lhsT layout, kept SBUF-resident),
the full x and h.T (replicated), and its shard of c.T. Matmuls run as
float32r (FP22 precision, 1 cycle/row at N=512 — same PE rate as bf16);
everything after the matmul (bias, activations, elementwise c/h update)
is fp32. No collectives: the host concatenates the 8 output shards.
"""

import numpy as np

N_CORES = 8
IN_SIZE = 2048
HIDDEN = 2048
BATCH = 4096
P = 128  # SBUF/PSUM partitions
NB = 512  # batch tile (matmul free dim; one PSUM bank of fp32)
G = 4  # gates: i, f, g, o


def build_lstm_nc(in_size, hid_size, shard, batch, nb=NB, reps=1, loop_reps=0):
    """Build + compile the Bass program (identical NEFF for every core).

    shard: hidden rows computed per core (M), multiple of 128.
    reps: statically repeat the whole compute in-NEFF (timing only).
    loop_reps: if >0, additionally wrap the compute in a hardware For_i
        loop with this trip count (timing only; outputs idempotent).
    """
    import concourse.bass as bass
    import concourse.tile as tile
    from concourse import bacc, mybir
    from concourse._compat import get_trn_type

    f32 = mybir.dt.float32
    f32r = mybir.dt.float32r
    AF = mybir.ActivationFunctionType
    gate_funcs = [AF.Sigmoid, AF.Sigmoid, AF.Tanh, AF.Sigmoid]

    assert shard % P == 0 and in_size % P == 0 and hid_size % P == 0
    assert batch % nb == 0
    m_tiles = shard // P
    nkx = in_size // P
    nkh = hid_size // P
    nn = batch // nb

    nc = bacc.Bacc(get_trn_type() or "TRN2", target_bir_lowering=False, debug=False)

    wx_d = nc.dram_tensor("wx", [in_size, G * shard], f32r, kind="ExternalInput")
    wh_d = nc.dram_tensor("wh", [hid_size, G * shard], f32r, kind="ExternalInput")
    x_d = nc.dram_tensor("x", [in_size, batch], f32r, kind="ExternalInput")
    ht_d = nc.dram_tensor("ht", [hid_size, batch], f32r, kind="ExternalInput")
    ct_d = nc.dram_tensor("ct", [shard, batch], f32, kind="ExternalInput")
    b_d = nc.dram_tensor("bias", [P, G * m_tiles], f32, kind="ExternalInput")
    ho_d = nc.dram_tensor("h_out", [shard, batch], f32, kind="ExternalOutput")
    co_d = nc.dram_tensor("c_out", [shard, batch], f32, kind="ExternalOutput")

    with tile.TileContext(nc) as tc:
        with (
            tc.tile_pool(name="wpool", bufs=1) as wpool,
            tc.tile_pool(name="xpool", bufs=4) as xpool,
            tc.tile_pool(name="hpool", bufs=4) as hpool,
            tc.tile_pool(name="cpool", bufs=3) as cpool,
            tc.tile_pool(name="gpool", bufs=2) as gpool,
            tc.tile_pool(name="tpool", bufs=2) as tpool,
            tc.tile_pool(name="bpool", bufs=1) as bpool,
            tc.tile_pool(name="psum", bufs=1, space=bass.MemorySpace.PSUM) as pspool,
        ):
            # Resident weights: one [128, 4*shard] tile per contraction slice.
            # Weight preload on the gpsimd (SWDGE) queue so the x/h tile
            # stream on the sync HWDGE ring isn't stuck behind 16MB of
            # weights at kernel start.
            wx_sb = []
            for k in range(nkx):
                wt = wpool.tile([P, G * shard], f32r, tag=f"wx{k}", name=f"wx{k}")
                nc.gpsimd.dma_start(out=wt[:], in_=wx_d[k * P : (k + 1) * P, :])
                wx_sb.append(wt)
            wh_sb = []
            for k in range(nkh):
                wt = wpool.tile([P, G * shard], f32r, tag=f"wh{k}", name=f"wh{k}")
                nc.gpsimd.dma_start(out=wt[:], in_=wh_d[k * P : (k + 1) * P, :])
                wh_sb.append(wt)
            bias_sb = bpool.tile([P, G * m_tiles], f32, name="bias_sb")
            nc.gpsimd.dma_start(out=bias_sb[:], in_=b_d[:])

            def emit_body():
              for rep in range(reps):
               for n in range(nn):
                ncol = slice(n * nb, (n + 1) * nb)
                # One PSUM bank per (gate, m): 4 * m_tiles <= 8 banks.
                ps = [
                    [
                        pspool.tile([P, nb], f32, tag=f"ps{g}_{m}", name=f"ps{g}_{m}_{n}_{rep}")
                        for m in range(m_tiles)
                    ]
                    for g in range(G)
                ]
                for kk in range(nkx + nkh):
                    if kk < nkx:
                        k = kk
                        rhs = xpool.tile([P, nb], f32r, tag="xt", name=f"xt{n}_{kk}")
                        nc.sync.dma_start(out=rhs[:], in_=x_d[k * P : (k + 1) * P, ncol])
                        w = wx_sb[k]
                    else:
                        k = kk - nkx
                        rhs = hpool.tile([P, nb], f32r, tag="htt", name=f"ht{n}_{kk}")
                        nc.sync.dma_start(out=rhs[:], in_=ht_d[k * P : (k + 1) * P, ncol])
                        w = wh_sb[k]
                    start = kk == 0
                    stop = kk == nkx + nkh - 1
                    for m in range(m_tiles):
                        for g in range(G):
                            nc.tensor.matmul(
                                ps[g][m][:],
                                w[:, g * shard + m * P : g * shard + (m + 1) * P],
                                rhs[:],
                                start=start,
                                stop=stop,
                            )
                for m in range(m_tiles):
                    mrow = slice(m * P, (m + 1) * P)
                    ct_t = cpool.tile([P, nb], f32, tag="ct", name=f"ct{n}_{m}")
                    nc.scalar.dma_start(out=ct_t[:], in_=ct_d[mrow, ncol])
                    gt = []
                    for g in range(G):
                        gs = gpool.tile([P, nb], f32, tag=f"g{g}", name=f"g{g}_{n}_{m}")
                        nc.scalar.activation(
                            gs[:],
                            ps[g][m][:],
                            gate_funcs[g],
                            bias=bias_sb[:, g * m_tiles + m : g * m_tiles + m + 1],
                        )
                        gt.append(gs)
                    i_t, f_t, g_t, o_t = gt
                    fc = tpool.tile([P, nb], f32, tag="fc", name=f"fc{n}_{m}")
                    nc.vector.tensor_mul(fc[:], f_t[:], ct_t[:])
                    ig = tpool.tile([P, nb], f32, tag="ig", name=f"ig{n}_{m}")
                    nc.vector.tensor_mul(ig[:], i_t[:], g_t[:])
                    cn = tpool.tile([P, nb], f32, tag="cn", name=f"cn{n}_{m}")
                    nc.vector.tensor_add(cn[:], fc[:], ig[:])
                    th = tpool.tile([P, nb], f32, tag="th", name=f"th{n}_{m}")
                    nc.scalar.activation(th[:], cn[:], AF.Tanh)
                    hn = tpool.tile([P, nb], f32, tag="hn", name=f"hn{n}_{m}")
                    nc.vector.tensor_mul(hn[:], o_t[:], th[:])
                    nc.scalar.dma_start(out=co_d[mrow, ncol], in_=cn[:])
                    nc.scalar.dma_start(out=ho_d[mrow, ncol], in_=hn[:])
                del ps

            if loop_reps > 0:
                with tc.For_i(0, loop_reps, 1):
                    emit_body()
            else:
                emit_body()

    nc.compile()
    return nc


_NC_CACHE = {}


def _get_nc(key, *args):
    if key not in _NC_CACHE:
        _NC_CACHE[key] = build_lstm_nc(*args)
    return _NC_CACHE[key]


def prepare_inputs(
    inputs, h, c,
    w_ii, w_if, w_ig, w_io,
    w_hi, w_hf, w_hg, w_ho,
    b_ii, b_hi, b_if, b_hf, b_ig, b_hg, b_io, b_ho,
    n_cores=N_CORES,
):
    """Host-side prep: per-core input maps for the SPMD kernel."""
    in_size, batch = inputs.shape
    hid = h.shape[1]
    shard = hid // n_cores
    m_tiles = shard // P

    x = np.ascontiguousarray(inputs, dtype=np.float32)
    ht = np.ascontiguousarray(h.T, dtype=np.float32)
    ct = np.ascontiguousarray(c.T, dtype=np.float32)

    w_in = [w_ii, w_if, w_ig, w_io]
    w_hid = [w_hi, w_hf, w_hg, w_ho]
    biases = [b_ii + b_hi, b_if + b_hf, b_ig + b_hg, b_io + b_ho]

    wxT = [np.ascontiguousarray(w.T, dtype=np.float32) for w in w_in]
    whT = [np.ascontiguousarray(w.T, dtype=np.float32) for w in w_hid]

    in_maps = []
    for s in range(n_cores):
        rows = slice(s * shard, (s + 1) * shard)
        wx_s = np.concatenate([w[:, rows] for w in wxT], axis=1)
        wh_s = np.concatenate([w[:, rows] for w in whT], axis=1)
        # bias_sb[p, g*m_tiles + m] = bias_g[s*shard + m*128 + p]
        b_cols = []
        for g in range(G):
            bg = np.asarray(biases[g], dtype=np.float32).reshape(-1)[rows]
            for m in range(m_tiles):
                b_cols.append(bg[m * P : (m + 1) * P])
        bias_s = np.ascontiguousarray(np.stack(b_cols, axis=1), dtype=np.float32)
        in_maps.append(
            {
                "wx": np.ascontiguousarray(wx_s),
                "wh": np.ascontiguousarray(wh_s),
                "x": x,
                "ht": ht,
                "ct": np.ascontiguousarray(ct[rows, :]),
                "bias": bias_s,
            }
        )
    return in_maps


def run_spmd(nc, in_maps, **kwargs):
    from concourse.bass_utils import run_bass_kernel_spmd

    return run_bass_kernel_spmd(nc, in_maps, core_ids=list(range(len(in_maps))), **kwargs)


def assemble_outputs(results):
    ht_next = np.concatenate([r["h_out"] for r in results], axis=0)
    ct_next = np.concatenate([r["c_out"] for r in results], axis=0)
    return ht_next.T, ct_next.T


def kernel(**inputs):
    in_maps = prepare_inputs(**{k: np.asarray(v) for k, v in inputs.items()})
    in_size, batch = inputs["inputs"].shape
    hid = inputs["h"].shape[1]
    shard = hid // N_CORES
    nc = _get_nc((in_size, hid, shard, batch), in_size, hid, shard, batch)
    res = run_spmd(nc, in_maps)
    return assemble_outputs(res.results)
